# revision 42
# baseline (speedup 1.0000x reference)
"""Trainium2 Bass kernel for nn_Pooling_block (B=128, N=785, C=384, pp=2).

Pure data-parallel over batch: 16 batches per core x 8 NeuronCores.

v3 design (memory-regime; DMA floor ~135us/core):
  - All DRAM inputs declared float32r (same bits as f32) so every load runs
    on HWDGE with no cast DMAs and feeds PE matmuls at full f32r rate.
  - x host-pre-gathered to patch-major [B, 196, 4C]; per batch two loads:
    patches 0:128 -> [128, 4C] (one 6 KB descriptor per partition, engages
    all 16 SDMA engines evenly) and patches 128:196 -> [68, 4C].
  - edge folded [128, 4C] (rows 0:512) + [64, 4C] at base partition 64
    (rows 512:768) + [17, C]: shifts edge bytes onto partitions 64-127 to
    balance per-partition DMA load against G (which is heavier on 0-67).
  - per-batch sums -> sigmoid -> s_row; s columns collected per GROUP of 4
    into one [128, 3, 4] PSUM tile; ci = s @ W_lin.T per batch (3 matmuls),
    broadcast to 128 partitions via gpsimd partition_broadcast.
  - scores: fused DVE scalar_tensor_tensor in f32 (no cast passes).
  - pooled emitted in bf16 -> bf16 PE transposes -> bf16 final matmuls
    against W_out_cls.T (K=384 fp32 accumulation; ~2e-3 rel err, gate 2e-2).
  - loads issue from SP only (its stream has no compute to block); stores
    from ACT right after its own PSUM->SBUF copies.
  - PSUM: 8 banks (es, ns, scolT, cip, tp2 x2, fo x2).
"""
import os
import sys

sys.path.insert(0, "/opt/trn_rl_repo")

import numpy as np

import concourse.bass as bass
import concourse.tile as tile
from concourse import bacc, mybir
from concourse.bass_utils import run_bass_kernel_spmd

B, N, C = 128, 785, 384
HW = N - 1          # 784
H = 28              # grid side
HP = 14             # pooled grid side
NPATCH = HP * HP    # 196
NB = 16             # batches per core
NCORES = 8
NOUT = 1 + NPATCH   # 197
CO = 2 * C          # 768
GRP = 2             # batches per chain group
NGRP = NB // GRP
R1, R2 = 128, 68    # patch split

F32 = mybir.dt.float32
F32R = mybir.dt.float32r
BF16 = mybir.dt.bfloat16
ADD = mybir.AluOpType.add
MUL = mybir.AluOpType.mult
SIGMOID = mybir.ActivationFunctionType.Sigmoid


def build_program(w_scalars):
    """Build the per-core SPMD program. w_scalars = (w00, w01, w10, w11) when
    the per-patch weights are channel-uniform, else None (general path)."""
    nc = bacc.Bacc(None, target_bir_lowering=False, debug=False)

    # x is host-pre-gathered patch-major and zero-padded to 256 patches so
    # BOTH per-batch loads span 128 partitions (sub-128-partition HWDGE
    # transfers concentrate on 4 of the 16 SDMA engines).
    x_d = nc.declare_dram_parameter("x", [NB, 256, 4 * C], F32R, isOutput=False)
    e_d = nc.declare_dram_parameter("edge", [NB, N, C], F32R, isOutput=False)
    wlt_d = nc.declare_dram_parameter("wlt", [C, C], F32R, isOutput=False)
    wct_d = nc.declare_dram_parameter("wct", [C, CO], F32, isOutput=False)
    id_d = nc.declare_dram_parameter("ident", [128, 128], F32R, isOutput=False)
    clsc_d = nc.declare_dram_parameter("cls_cm", [128, 3, NB], F32, isOutput=False)
    if w_scalars is None:
        wqr_d = nc.declare_dram_parameter("wqr", [4, 128, C], F32, isOutput=False)
    out_d = nc.declare_dram_parameter("out", [NB, NOUT, CO], F32, isOutput=True)

    uniform_w = w_scalars is not None and len(set(w_scalars)) == 1
    # (tile index, rows, acm column offset)
    SPLITS = ((0, R1, 1), (1, R2, 1 + R1))

    with tile.TileContext(nc) as tc:
        with (
            tc.tile_pool(name="const", bufs=1) as cpool,
            tc.tile_pool(name="gx", bufs=7) as gxp,
            tc.tile_pool(name="ed", bufs=3) as edp,
            tc.tile_pool(name="apool", bufs=6) as ap,
            tc.tile_pool(name="work", bufs=2) as wk,
            tc.tile_pool(name="small", bufs=2) as sm,
            tc.tile_pool(name="cibp", bufs=1) as cibp,
            tc.tile_pool(name="acm", bufs=2) as acmp,
            tc.tile_pool(name="ost", bufs=2) as ostp,
            tc.tile_pool(name="psE", bufs=1, space="PSUM") as psE,
            tc.tile_pool(name="psC", bufs=1, space="PSUM") as psC,
            tc.tile_pool(name="psT", bufs=2, space="PSUM") as psT,
            tc.tile_pool(name="psF", bufs=2, space="PSUM") as psF,
        ):
            # ---- constants ----
            ones_f = cpool.tile([128, 1], F32)
            nc.vector.memset(ones_f[:], 1.0)
            ones_r = cpool.tile([128, 1], F32R)
            nc.vector.tensor_copy(ones_r[:], ones_f[:])

            ident_r = cpool.tile([128, 128], F32R)
            nc.sync.dma_start(ident_r[:], id_d[:])
            ident_bf = cpool.tile([128, 128], BF16)
            nc.vector.tensor_copy(ident_bf[:], ident_r[:])

            wlt_r = []
            for cch in range(3):
                t = cpool.tile([128, C], F32R, tag=f"wlt{cch}")
                nc.sync.dma_start(t[:], wlt_d[128 * cch : 128 * (cch + 1), :])
                wlt_r.append(t)

            wct_bf = []
            for cch in range(3):
                stg = ostp.tile([128, CO], F32, tag="ost0")
                nc.sync.dma_start(stg[:], wct_d[128 * cch : 128 * (cch + 1), :])
                t = cpool.tile([128, CO], BF16, tag=f"wct{cch}")
                nc.vector.tensor_copy(t[:], stg[:])
                wct_bf.append(t)

            if w_scalars is None:
                wqr_t = []
                for k in range(4):
                    t = cpool.tile([128, C], F32, tag=f"wqr{k}")
                    nc.sync.dma_start(t[:], wqr_d[k])
                    wqr_t.append(t)

            cls_cm = cpool.tile([128, 3, NB], F32)
            nc.sync.dma_start(cls_cm[:], clsc_d[:])
            cls_bf = cpool.tile([128, 3, NB], BF16)
            nc.vector.tensor_copy(cls_bf[:], cls_cm[:])

            group_list = [range(s, s + GRP) for s in range(0, NB, GRP)]
            for bs in group_list:
                glen = len(bs)
                g_t, a_t = {}, {}

                # -- sub-loop 1: loads + per-batch token sums --
                scolT = psC.tile([128, 3, GRP], F32, tag="scolT")
                for b in bs:
                    gb = b - bs[0]
                    # all loads issue from SP (sync): its stream has no
                    # compute, so slot waits never block compute instructions.
                    g1 = gxp.tile([R1, 4, C], F32R, tag="g1")
                    nc.sync.dma_start(
                        g1[:], x_d[b, 0:128, :].rearrange("p (k c) -> p k c", k=4)
                    )
                    # g2 loads 128 padded patches; compute uses rows 0:68 only
                    g2f = gxp.tile([128, 4, C], F32R, tag="g2")
                    nc.sync.dma_start(
                        g2f[:], x_d[b, 128:256, :].rearrange("p (k c) -> p k c", k=4)
                    )
                    g_t[(b, 0)], g_t[(b, 1)] = g1, g2f

                    efold = edp.tile([128, 6 * C], F32R, tag="efold")
                    nc.sync.dma_start(
                        efold[:],
                        e_d[b, 0:768, :].rearrange("(p k) c -> p (k c)", p=128),
                    )
                    etl = edp.tile([17, C], F32R, tag="etl")
                    nc.gpsimd.dma_start(etl[:], e_d[b, 768:785, :])

                    # edge sums
                    es = psE.tile([1, C], F32, tag="es")
                    for k in range(6):
                        nc.tensor.matmul(
                            es[:], ones_r[:], efold[:, C * k : C * (k + 1)],
                            start=(k == 0), stop=False,
                        )
                    nc.tensor.matmul(
                        es[:], ones_r[0:17, :], etl[:], start=False, stop=True
                    )

                    # vertical pair sums A_q (f32r) + node sums
                    ns = psE.tile([1, C], F32, tag="ns")
                    first = True
                    for t_i, rn, _ in SPLITS:
                        at = ap.tile([rn, 2, C], F32R, tag=f"a{t_i}")
                        for q in range(2):
                            nc.vector.tensor_add(
                                at[:, q, :],
                                g_t[(b, t_i)][0:rn, q, :],
                                g_t[(b, t_i)][0:rn, 2 + q, :],
                            )
                        a_t[(b, t_i)] = at
                    for q in range(2):
                        for t_i, rn, _ in SPLITS:
                            nc.tensor.matmul(
                                ns[:], ones_r[0:rn, :], a_t[(b, t_i)][:, q, :],
                                start=first, stop=(q == 1 and t_i == 1),
                            )
                            first = False

                    # sigmoid means -> s_row; transpose into column gb of scolT
                    se = sm.tile([1, C], F32, tag="se")
                    nc.scalar.activation(se[:], es[:], SIGMOID, scale=1.0 / N)
                    sn = sm.tile([1, C], F32, tag="sn")
                    nc.scalar.activation(sn[:], ns[:], SIGMOID, scale=1.0 / HW)
                    s_row = sm.tile([1, C], F32, tag="srow")
                    nc.vector.tensor_add(s_row[:], se[:], sn[:])
                    for cch in range(3):
                        nc.tensor.matmul(
                            scolT[:, cch, gb : gb + 1],
                            s_row[:, 128 * cch : 128 * (cch + 1)],
                            ones_f[0:1, :],
                            start=True, stop=True,
                        )

                # -- group chain: s columns -> ci rows -> broadcasts up front --
                scolT_sb = sm.tile([128, 3, GRP], F32R, tag="scolsb")
                nc.scalar.copy(scolT_sb[:], scolT[:])

                cibs = []
                for gb in range(glen):
                    cip = psC.tile([1, C], F32, tag="cip")
                    for cch in range(3):
                        nc.tensor.matmul(
                            cip[:], scolT_sb[:, cch, gb : gb + 1], wlt_r[cch][:],
                            start=(cch == 0), stop=(cch == 2),
                        )
                    ci_b = sm.tile([1, C], F32R, tag=f"cirow{gb}")
                    nc.scalar.copy(ci_b[:], cip[:])
                    cib = cibp.tile([128, C], F32R, tag=f"cib{gb}")
                    nc.gpsimd.partition_broadcast(cib[:], ci_b[:])
                    cibs.append(cib)

                # -- sub-loop 2: scores / pooled / transpose / final / store --
                for b in bs:
                    gb = b - bs[0]
                    cib = cibs[gb]

                    sig = {}
                    for t_i, rn, _ in SPLITS:
                        sa = sm.tile([rn, 4], F32, tag=f"sacc{t_i}")
                        for k in range(4):
                            scr = wk.tile([rn, C], BF16, tag=f"scr{t_i}")
                            nc.vector.scalar_tensor_tensor(
                                scr[:], g_t[(b, t_i)][0:rn, k, :], 1.0, cib[0:rn, :],
                                MUL, MUL, accum_out=sa[:, k : k + 1],
                            )
                        sg = sm.tile([rn, 4], F32, tag=f"sig{t_i}")
                        nc.scalar.activation(sg[:], sa[:], SIGMOID)
                        sig[t_i] = sg

                    pooled = {}
                    for t_i, rn, _ in SPLITS:
                        sp = sm.tile([rn, 4], F32, tag=f"sp{t_i}")
                        nc.vector.tensor_scalar_add(sp[:], sig[t_i][:], 1.0)
                        at = a_t[(b, t_i)]
                        if w_scalars is not None:
                            w00 = w_scalars[0]
                            if not uniform_w:
                                wrow = sm.tile([rn, 4], F32, tag=f"wrow{t_i}")
                                for k in range(4):
                                    nc.vector.memset(
                                        wrow[:, k : k + 1], float(w_scalars[k])
                                    )
                                nc.vector.tensor_mul(sp[:], sp[:], wrow[:])
                            t01 = sm.tile([rn, 2], F32, tag=f"t01_{t_i}")
                            nc.vector.tensor_add(t01[:], sp[:, 0:4:2], sp[:, 1:4:2])
                            if uniform_w and w00 != 1.0:
                                nc.vector.tensor_scalar_mul(t01[:], t01[:], float(w00))
                            p0 = wk.tile([rn, C], F32, tag=f"p0_{t_i}")
                            nc.vector.tensor_scalar_mul(p0[:], at[:, 0, :], t01[:, 0:1])
                            pl = wk.tile([rn, C], BF16, tag=f"pl{t_i}")
                            nc.vector.scalar_tensor_tensor(
                                pl[:], at[:, 1, :], t01[:, 1:2], p0[:], MUL, ADD
                            )
                        else:
                            # general per-channel weights: m_q[rn, C], combine
                            mqs = []
                            for q in range(2):
                                m0 = wk.tile([rn, C], F32, tag=f"mq{q}{t_i}a")
                                nc.vector.tensor_scalar_mul(
                                    m0[:], wqr_t[2 * q][0:rn, :],
                                    sp[:, 2 * q : 2 * q + 1],
                                )
                                mq = wk.tile([rn, C], F32, tag=f"mq{q}{t_i}b")
                                nc.vector.scalar_tensor_tensor(
                                    mq[:], wqr_t[2 * q + 1][0:rn, :],
                                    sp[:, 2 * q + 1 : 2 * q + 2], m0[:], MUL, ADD,
                                )
                                mqs.append(mq)
                            p0 = wk.tile([rn, C], F32, tag=f"p0_{t_i}")
                            nc.vector.tensor_mul(p0[:], at[:, 0, :], mqs[0][:])
                            p1 = wk.tile([rn, C], F32, tag=f"p1_{t_i}")
                            nc.vector.tensor_mul(p1[:], at[:, 1, :], mqs[1][:])
                            pl = wk.tile([rn, C], BF16, tag=f"pl{t_i}")
                            nc.vector.tensor_add(pl[:], p0[:], p1[:])
                        pooled[t_i] = pl

                    # c-major A via bf16 PE transposes + cls column
                    a_cm = []
                    for cch in range(3):
                        tp2 = psT.tile([128, NPATCH], BF16, tag="tp2")
                        for t_i, rn, col in SPLITS:
                            nc.tensor.transpose(
                                tp2[:, col - 1 : col - 1 + rn],
                                pooled[t_i][:, 128 * cch : 128 * (cch + 1)],
                                ident_bf[0:rn, 0:rn],
                            )
                        acm = acmp.tile([128, NOUT], BF16, tag=f"acm{cch}")
                        nc.scalar.copy(acm[:, 0:1], cls_bf[:, cch, b : b + 1])
                        nc.scalar.copy(acm[:, 1:NOUT], tp2[:])
                        a_cm.append(acm)

                    # final matmul: out[row, co] = A_cm.T @ W_out_cls.T (bf16)
                    for rch, (r0, rn) in enumerate(((0, 128), (128, 69))):
                        stile = ostp.tile([128, CO], F32, tag=f"ost{rch}")
                        for nh in range(2):
                            fo = psF.tile([128, C], F32, tag="fo")
                            for cch in range(3):
                                nc.tensor.matmul(
                                    fo[0:rn, :],
                                    a_cm[cch][:, r0 : r0 + rn],
                                    wct_bf[cch][:, C * nh : C * (nh + 1)],
                                    start=(cch == 0), stop=(cch == 2),
                                )
                            # both halves on ACT so the store that follows in
                            # ACT's stream never waits on another engine
                            nc.scalar.copy(
                                stile[0:rn, C * nh : C * (nh + 1)], fo[0:rn, :]
                            )
                        # stores via SWDGE: its descriptor swizzle spreads
                        # partitions across all 16 SDMA engines
                        nc.gpsimd.dma_start(out_d[b, r0 : r0 + rn, :], stile[0:rn, :])

    nc.compile()
    return nc


def prepare(x, edge, W_lin, W_out_cls, weights):
    """Host-side prep shared by kernel() and the timing harness: returns
    (w_scalars, in_maps)."""
    x = np.ascontiguousarray(x, dtype=np.float32)
    edge = np.ascontiguousarray(edge, dtype=np.float32)
    # pre-gather nodes to patch-major [B, 196, 4C] (slot = 2p+q), zero-padded
    # to 256 patches so both per-batch loads span 128 partitions
    xg = np.zeros((B, 256, 4 * C), dtype=np.float32)
    xg[:, :NPATCH] = (
        x[:, 1:, :]
        .reshape(B, HP, 2, HP, 2, C)
        .transpose(0, 1, 3, 2, 4, 5)
        .reshape(B, NPATCH, 4 * C)
    )
    wlt = np.ascontiguousarray(np.asarray(W_lin).T, dtype=np.float32)
    wct = np.ascontiguousarray(np.asarray(W_out_cls).T, dtype=np.float32)
    w = np.asarray(weights, dtype=np.float32)

    c_uniform = bool(np.all(w == w[0:1]))
    w_scalars = tuple(float(v) for v in w[0].reshape(4)) if c_uniform else None

    ident = np.eye(128, dtype=np.float32)
    in_maps = []
    for core in range(NCORES):
        sl = slice(core * NB, (core + 1) * NB)
        cls_cm = np.ascontiguousarray(
            x[sl, 0, :].T.reshape(3, 128, NB).transpose(1, 0, 2), dtype=np.float32
        )
        m = {
            "x": xg[sl], "edge": edge[sl], "wlt": wlt, "wct": wct, "ident": ident,
            "cls_cm": cls_cm,
        }
        if w_scalars is None:
            wqr = np.empty((4, 128, C), dtype=np.float32)
            for q in range(2):
                for r in range(2):
                    wqr[2 * q + r] = np.broadcast_to(w[:, q, r], (128, C))
            m["wqr"] = wqr
        in_maps.append(m)
    return w_scalars, in_maps


def kernel(x, edge, W_lin, W_out_cls, weights):
    w_scalars, in_maps = prepare(x, edge, W_lin, W_out_cls, weights)
    nc = build_program(w_scalars)
    res = run_bass_kernel_spmd(nc, in_maps, list(range(NCORES)))
    out = np.concatenate([r["out"] for r in res.results], axis=0)
    return out


# revision 44
# speedup vs baseline: 1.0623x; 1.0623x over previous
"""Trainium2 Bass kernel for nn_Pooling_block (B=128, N=785, C=384, pp=2).

Pure data-parallel over batch: 16 batches per core x 8 NeuronCores.

v3 design (memory-regime; DMA floor ~135us/core):
  - All DRAM inputs declared float32r (same bits as f32) so every load runs
    on HWDGE with no cast DMAs and feeds PE matmuls at full f32r rate.
  - x host-pre-gathered to patch-major [B, 196, 4C]; per batch two loads:
    patches 0:128 -> [128, 4C] (one 6 KB descriptor per partition, engages
    all 16 SDMA engines evenly) and patches 128:196 -> [68, 4C].
  - edge folded [128, 4C] (rows 0:512) + [64, 4C] at base partition 64
    (rows 512:768) + [17, C]: shifts edge bytes onto partitions 64-127 to
    balance per-partition DMA load against G (which is heavier on 0-67).
  - per-batch sums -> sigmoid -> s_row; s columns collected per GROUP of 4
    into one [128, 3, 4] PSUM tile; ci = s @ W_lin.T per batch (3 matmuls),
    broadcast to 128 partitions via gpsimd partition_broadcast.
  - scores: fused DVE scalar_tensor_tensor in f32 (no cast passes).
  - pooled emitted in bf16 -> bf16 PE transposes -> bf16 final matmuls
    against W_out_cls.T (K=384 fp32 accumulation; ~2e-3 rel err, gate 2e-2).
  - loads issue from SP only (its stream has no compute to block); stores
    from ACT right after its own PSUM->SBUF copies.
  - PSUM: 8 banks (es, ns, scolT, cip, tp2 x2, fo x2).
"""
import os
import sys

sys.path.insert(0, "/opt/trn_rl_repo")

import numpy as np

import concourse.bass as bass
import concourse.tile as tile
from concourse import bacc, mybir
from concourse.bass_utils import run_bass_kernel_spmd

B, N, C = 128, 785, 384
HW = N - 1          # 784
H = 28              # grid side
HP = 14             # pooled grid side
NPATCH = HP * HP    # 196
NB = 16             # batches per core
NCORES = 8
NOUT = 1 + NPATCH   # 197
CO = 2 * C          # 768
GRP = 4             # batches per chain group
NGRP = NB // GRP
R1, R2 = 128, 68    # patch split

F32 = mybir.dt.float32
F32R = mybir.dt.float32r
BF16 = mybir.dt.bfloat16
ADD = mybir.AluOpType.add
MUL = mybir.AluOpType.mult
SIGMOID = mybir.ActivationFunctionType.Sigmoid


def build_program(w_scalars):
    """Build the per-core SPMD program. w_scalars = (w00, w01, w10, w11) when
    the per-patch weights are channel-uniform, else None (general path)."""
    nc = bacc.Bacc(None, target_bir_lowering=False, debug=False)

    # x is host-pre-gathered patch-major and zero-padded to 256 patches so
    # BOTH per-batch loads span 128 partitions (sub-128-partition HWDGE
    # transfers concentrate on 4 of the 16 SDMA engines).
    x_d = nc.declare_dram_parameter("x", [NB, 256, 4 * C], F32R, isOutput=False)
    e_d = nc.declare_dram_parameter("edge", [NB, N, C], F32R, isOutput=False)
    wlt_d = nc.declare_dram_parameter("wlt", [C, C], F32R, isOutput=False)
    wct_d = nc.declare_dram_parameter("wct", [C, CO], F32, isOutput=False)
    id_d = nc.declare_dram_parameter("ident", [128, 128], F32R, isOutput=False)
    clsc_d = nc.declare_dram_parameter("cls_cm", [128, 3, NB], F32, isOutput=False)
    if w_scalars is None:
        wqr_d = nc.declare_dram_parameter("wqr", [4, 128, C], F32, isOutput=False)
    out_d = nc.declare_dram_parameter("out", [NB, NOUT, CO], F32, isOutput=True)

    uniform_w = w_scalars is not None and len(set(w_scalars)) == 1
    # (tile index, rows, acm column offset)
    SPLITS = ((0, R1, 1), (1, R2, 1 + R1))

    with tile.TileContext(nc) as tc:
        with (
            tc.tile_pool(name="const", bufs=1) as cpool,
            tc.tile_pool(name="gx", bufs=7) as gxp,
            tc.tile_pool(name="ed", bufs=2) as edp,
            tc.tile_pool(name="apool", bufs=6) as ap,
            tc.tile_pool(name="work", bufs=2) as wk,
            tc.tile_pool(name="small", bufs=2) as sm,
            tc.tile_pool(name="cibp", bufs=1) as cibp,
            tc.tile_pool(name="acm", bufs=2) as acmp,
            tc.tile_pool(name="ost", bufs=2) as ostp,
            tc.tile_pool(name="psE", bufs=1, space="PSUM") as psE,
            tc.tile_pool(name="psC", bufs=1, space="PSUM") as psC,
            tc.tile_pool(name="psT", bufs=2, space="PSUM") as psT,
            tc.tile_pool(name="psF", bufs=2, space="PSUM") as psF,
        ):
            # ---- constants ----
            ones_f = cpool.tile([128, 1], F32)
            nc.vector.memset(ones_f[:], 1.0)
            ones_r = cpool.tile([128, 1], F32R)
            nc.vector.tensor_copy(ones_r[:], ones_f[:])

            ident_r = cpool.tile([128, 128], F32R)
            nc.sync.dma_start(ident_r[:], id_d[:])
            ident_bf = cpool.tile([128, 128], BF16)
            nc.vector.tensor_copy(ident_bf[:], ident_r[:])

            wlt_r = []
            for cch in range(3):
                t = cpool.tile([128, C], F32R, tag=f"wlt{cch}")
                nc.sync.dma_start(t[:], wlt_d[128 * cch : 128 * (cch + 1), :])
                wlt_r.append(t)

            wct_bf = []
            for cch in range(3):
                stg = ostp.tile([128, CO], F32, tag="ost0")
                nc.sync.dma_start(stg[:], wct_d[128 * cch : 128 * (cch + 1), :])
                t = cpool.tile([128, CO], BF16, tag=f"wct{cch}")
                nc.vector.tensor_copy(t[:], stg[:])
                wct_bf.append(t)

            if w_scalars is None:
                wqr_t = []
                for k in range(4):
                    t = cpool.tile([128, C], F32, tag=f"wqr{k}")
                    nc.sync.dma_start(t[:], wqr_d[k])
                    wqr_t.append(t)

            cls_cm = cpool.tile([128, 3, NB], F32)
            nc.sync.dma_start(cls_cm[:], clsc_d[:])
            cls_bf = cpool.tile([128, 3, NB], BF16)
            nc.vector.tensor_copy(cls_bf[:], cls_cm[:])

            group_list = [range(s, s + GRP) for s in range(0, NB, GRP)]
            for bs in group_list:
                glen = len(bs)
                g_t, a_t = {}, {}

                # -- sub-loop 1: loads + per-batch token sums --
                scolT = psC.tile([128, 3, GRP], F32, tag="scolT")
                for b in bs:
                    gb = b - bs[0]
                    # all loads issue from SP (sync): its stream has no
                    # compute, so slot waits never block compute instructions.
                    g1 = gxp.tile([R1, 4, C], F32R, tag="g1")
                    nc.sync.dma_start(
                        g1[:], x_d[b, 0:128, :].rearrange("p (k c) -> p k c", k=4)
                    )
                    # g2 loads 128 padded patches; compute uses rows 0:68 only
                    g2f = gxp.tile([128, 4, C], F32R, tag="g2")
                    nc.sync.dma_start(
                        g2f[:], x_d[b, 128:256, :].rearrange("p (k c) -> p k c", k=4)
                    )
                    g_t[(b, 0)], g_t[(b, 1)] = g1, g2f

                    efold = edp.tile([128, 6 * C], F32R, tag="efold")
                    nc.sync.dma_start(
                        efold[:],
                        e_d[b, 0:768, :].rearrange("(p k) c -> p (k c)", p=128),
                    )
                    etl = edp.tile([17, C], F32R, tag="etl")
                    nc.gpsimd.dma_start(etl[:], e_d[b, 768:785, :])

                    # edge sums
                    es = psE.tile([1, C], F32, tag="es")
                    for k in range(6):
                        nc.tensor.matmul(
                            es[:], ones_r[:], efold[:, C * k : C * (k + 1)],
                            start=(k == 0), stop=False,
                        )
                    nc.tensor.matmul(
                        es[:], ones_r[0:17, :], etl[:], start=False, stop=True
                    )

                    # vertical pair sums A_q (f32r) + node sums
                    ns = psE.tile([1, C], F32, tag="ns")
                    first = True
                    for t_i, rn, _ in SPLITS:
                        at = ap.tile([rn, 2, C], F32R, tag=f"a{t_i}")
                        for q in range(2):
                            nc.vector.tensor_add(
                                at[:, q, :],
                                g_t[(b, t_i)][0:rn, q, :],
                                g_t[(b, t_i)][0:rn, 2 + q, :],
                            )
                        a_t[(b, t_i)] = at
                    for q in range(2):
                        for t_i, rn, _ in SPLITS:
                            nc.tensor.matmul(
                                ns[:], ones_r[0:rn, :], a_t[(b, t_i)][:, q, :],
                                start=first, stop=(q == 1 and t_i == 1),
                            )
                            first = False

                    # sigmoid means -> s_row; transpose into column gb of scolT
                    se = sm.tile([1, C], F32, tag="se")
                    nc.scalar.activation(se[:], es[:], SIGMOID, scale=1.0 / N)
                    sn = sm.tile([1, C], F32, tag="sn")
                    nc.scalar.activation(sn[:], ns[:], SIGMOID, scale=1.0 / HW)
                    s_row = sm.tile([1, C], F32, tag="srow")
                    nc.vector.tensor_add(s_row[:], se[:], sn[:])
                    for cch in range(3):
                        nc.tensor.matmul(
                            scolT[:, cch, gb : gb + 1],
                            s_row[:, 128 * cch : 128 * (cch + 1)],
                            ones_f[0:1, :],
                            start=True, stop=True,
                        )

                # -- group chain: s columns -> ci rows -> broadcasts up front --
                scolT_sb = sm.tile([128, 3, GRP], F32R, tag="scolsb")
                nc.scalar.copy(scolT_sb[:], scolT[:])

                cibs = []
                for gb in range(glen):
                    cip = psC.tile([1, C], F32, tag="cip")
                    for cch in range(3):
                        nc.tensor.matmul(
                            cip[:], scolT_sb[:, cch, gb : gb + 1], wlt_r[cch][:],
                            start=(cch == 0), stop=(cch == 2),
                        )
                    ci_b = sm.tile([1, C], F32R, tag=f"cirow{gb}")
                    nc.scalar.copy(ci_b[:], cip[:])
                    cib = cibp.tile([128, C], F32R, tag=f"cib{gb}")
                    nc.gpsimd.partition_broadcast(cib[:], ci_b[:])
                    cibs.append(cib)

                # -- sub-loop 2: scores / pooled / transpose / final / store --
                for b in bs:
                    gb = b - bs[0]
                    cib = cibs[gb]

                    sig = {}
                    for t_i, rn, _ in SPLITS:
                        sa = sm.tile([rn, 4], F32, tag=f"sacc{t_i}")
                        for k in range(4):
                            scr = wk.tile([rn, C], BF16, tag=f"scr{t_i}")
                            nc.vector.scalar_tensor_tensor(
                                scr[:], g_t[(b, t_i)][0:rn, k, :], 1.0, cib[0:rn, :],
                                MUL, MUL, accum_out=sa[:, k : k + 1],
                            )
                        sg = sm.tile([rn, 4], F32, tag=f"sig{t_i}")
                        nc.scalar.activation(sg[:], sa[:], SIGMOID)
                        sig[t_i] = sg

                    pooled = {}
                    for t_i, rn, _ in SPLITS:
                        sp = sm.tile([rn, 4], F32, tag=f"sp{t_i}")
                        nc.vector.tensor_scalar_add(sp[:], sig[t_i][:], 1.0)
                        at = a_t[(b, t_i)]
                        if w_scalars is not None:
                            w00 = w_scalars[0]
                            if not uniform_w:
                                wrow = sm.tile([rn, 4], F32, tag=f"wrow{t_i}")
                                for k in range(4):
                                    nc.vector.memset(
                                        wrow[:, k : k + 1], float(w_scalars[k])
                                    )
                                nc.vector.tensor_mul(sp[:], sp[:], wrow[:])
                            t01 = sm.tile([rn, 2], F32, tag=f"t01_{t_i}")
                            nc.vector.tensor_add(t01[:], sp[:, 0:4:2], sp[:, 1:4:2])
                            if uniform_w and w00 != 1.0:
                                nc.vector.tensor_scalar_mul(t01[:], t01[:], float(w00))
                            p0 = wk.tile([rn, C], F32, tag=f"p0_{t_i}")
                            nc.vector.tensor_scalar_mul(p0[:], at[:, 0, :], t01[:, 0:1])
                            pl = wk.tile([rn, C], BF16, tag=f"pl{t_i}")
                            nc.vector.scalar_tensor_tensor(
                                pl[:], at[:, 1, :], t01[:, 1:2], p0[:], MUL, ADD
                            )
                        else:
                            # general per-channel weights: m_q[rn, C], combine
                            mqs = []
                            for q in range(2):
                                m0 = wk.tile([rn, C], F32, tag=f"mq{q}{t_i}a")
                                nc.vector.tensor_scalar_mul(
                                    m0[:], wqr_t[2 * q][0:rn, :],
                                    sp[:, 2 * q : 2 * q + 1],
                                )
                                mq = wk.tile([rn, C], F32, tag=f"mq{q}{t_i}b")
                                nc.vector.scalar_tensor_tensor(
                                    mq[:], wqr_t[2 * q + 1][0:rn, :],
                                    sp[:, 2 * q + 1 : 2 * q + 2], m0[:], MUL, ADD,
                                )
                                mqs.append(mq)
                            p0 = wk.tile([rn, C], F32, tag=f"p0_{t_i}")
                            nc.vector.tensor_mul(p0[:], at[:, 0, :], mqs[0][:])
                            p1 = wk.tile([rn, C], F32, tag=f"p1_{t_i}")
                            nc.vector.tensor_mul(p1[:], at[:, 1, :], mqs[1][:])
                            pl = wk.tile([rn, C], BF16, tag=f"pl{t_i}")
                            nc.vector.tensor_add(pl[:], p0[:], p1[:])
                        pooled[t_i] = pl

                    # c-major A via bf16 PE transposes + cls column
                    a_cm = []
                    for cch in range(3):
                        tp2 = psT.tile([128, NPATCH], BF16, tag="tp2")
                        for t_i, rn, col in SPLITS:
                            nc.tensor.transpose(
                                tp2[:, col - 1 : col - 1 + rn],
                                pooled[t_i][:, 128 * cch : 128 * (cch + 1)],
                                ident_bf[0:rn, 0:rn],
                            )
                        acm = acmp.tile([128, NOUT], BF16, tag=f"acm{cch}")
                        nc.scalar.copy(acm[:, 0:1], cls_bf[:, cch, b : b + 1])
                        nc.scalar.copy(acm[:, 1:NOUT], tp2[:])
                        a_cm.append(acm)

                    # final matmul: out[row, co] = A_cm.T @ W_out_cls.T (bf16)
                    for rch, (r0, rn) in enumerate(((0, 128), (128, 69))):
                        stile = ostp.tile([128, CO], F32, tag=f"ost{rch}")
                        for nh in range(2):
                            fo = psF.tile([128, C], F32, tag="fo")
                            for cch in range(3):
                                nc.tensor.matmul(
                                    fo[0:rn, :],
                                    a_cm[cch][:, r0 : r0 + rn],
                                    wct_bf[cch][:, C * nh : C * (nh + 1)],
                                    start=(cch == 0), stop=(cch == 2),
                                )
                            # both halves on ACT so the store that follows in
                            # ACT's stream never waits on another engine
                            nc.scalar.copy(
                                stile[0:rn, C * nh : C * (nh + 1)], fo[0:rn, :]
                            )
                        # stores via SWDGE: its descriptor swizzle spreads
                        # partitions across all 16 SDMA engines
                        nc.gpsimd.dma_start(out_d[b, r0 : r0 + rn, :], stile[0:rn, :])

    nc.compile()
    return nc


def prepare(x, edge, W_lin, W_out_cls, weights):
    """Host-side prep shared by kernel() and the timing harness: returns
    (w_scalars, in_maps)."""
    x = np.ascontiguousarray(x, dtype=np.float32)
    edge = np.ascontiguousarray(edge, dtype=np.float32)
    # pre-gather nodes to patch-major [B, 196, 4C] (slot = 2p+q), zero-padded
    # to 256 patches so both per-batch loads span 128 partitions
    xg = np.zeros((B, 256, 4 * C), dtype=np.float32)
    xg[:, :NPATCH] = (
        x[:, 1:, :]
        .reshape(B, HP, 2, HP, 2, C)
        .transpose(0, 1, 3, 2, 4, 5)
        .reshape(B, NPATCH, 4 * C)
    )
    wlt = np.ascontiguousarray(np.asarray(W_lin).T, dtype=np.float32)
    wct = np.ascontiguousarray(np.asarray(W_out_cls).T, dtype=np.float32)
    w = np.asarray(weights, dtype=np.float32)

    c_uniform = bool(np.all(w == w[0:1]))
    w_scalars = tuple(float(v) for v in w[0].reshape(4)) if c_uniform else None

    ident = np.eye(128, dtype=np.float32)
    in_maps = []
    for core in range(NCORES):
        sl = slice(core * NB, (core + 1) * NB)
        cls_cm = np.ascontiguousarray(
            x[sl, 0, :].T.reshape(3, 128, NB).transpose(1, 0, 2), dtype=np.float32
        )
        m = {
            "x": xg[sl], "edge": edge[sl], "wlt": wlt, "wct": wct, "ident": ident,
            "cls_cm": cls_cm,
        }
        if w_scalars is None:
            wqr = np.empty((4, 128, C), dtype=np.float32)
            for q in range(2):
                for r in range(2):
                    wqr[2 * q + r] = np.broadcast_to(w[:, q, r], (128, C))
            m["wqr"] = wqr
        in_maps.append(m)
    return w_scalars, in_maps


def kernel(x, edge, W_lin, W_out_cls, weights):
    w_scalars, in_maps = prepare(x, edge, W_lin, W_out_cls, weights)
    nc = build_program(w_scalars)
    res = run_bass_kernel_spmd(nc, in_maps, list(range(NCORES)))
    out = np.concatenate([r["out"] for r in res.results], axis=0)
    return out


# revision 49
# speedup vs baseline: 1.0816x; 1.0182x over previous
"""Trainium2 Bass kernel for nn_Pooling_block (B=128, N=785, C=384, pp=2).

Pure data-parallel over batch: 16 batches per core x 8 NeuronCores.

v3 design (memory-regime; DMA floor ~135us/core):
  - All DRAM inputs declared float32r (same bits as f32) so every load runs
    on HWDGE with no cast DMAs and feeds PE matmuls at full f32r rate.
  - x host-pre-gathered to patch-major [B, 196, 4C]; per batch two loads:
    patches 0:128 -> [128, 4C] (one 6 KB descriptor per partition, engages
    all 16 SDMA engines evenly) and patches 128:196 -> [68, 4C].
  - edge folded [128, 4C] (rows 0:512) + [64, 4C] at base partition 64
    (rows 512:768) + [17, C]: shifts edge bytes onto partitions 64-127 to
    balance per-partition DMA load against G (which is heavier on 0-67).
  - per-batch sums -> sigmoid -> s_row; s columns collected per GROUP of 4
    into one [128, 3, 4] PSUM tile; ci = s @ W_lin.T per batch (3 matmuls),
    broadcast to 128 partitions via gpsimd partition_broadcast.
  - scores: fused DVE scalar_tensor_tensor in f32 (no cast passes).
  - pooled emitted in bf16 -> bf16 PE transposes -> bf16 final matmuls
    against W_out_cls.T (K=384 fp32 accumulation; ~2e-3 rel err, gate 2e-2).
  - loads issue from SP only (its stream has no compute to block); stores
    from ACT right after its own PSUM->SBUF copies.
  - PSUM: 8 banks (es, ns, scolT, cip, tp2 x2, fo x2).
"""
import os
import sys

sys.path.insert(0, "/opt/trn_rl_repo")

import numpy as np

import concourse.bass as bass
import concourse.tile as tile
from concourse import bacc, mybir
from concourse.bass_utils import run_bass_kernel_spmd

B, N, C = 128, 785, 384
HW = N - 1          # 784
H = 28              # grid side
HP = 14             # pooled grid side
NPATCH = HP * HP    # 196
NB = 16             # batches per core
NCORES = 8
NOUT = 1 + NPATCH   # 197
CO = 2 * C          # 768
GRP = 4             # batches per chain group
NGRP = NB // GRP
R1, R2 = 128, 68    # patch split

F32 = mybir.dt.float32
F32R = mybir.dt.float32r
BF16 = mybir.dt.bfloat16
ADD = mybir.AluOpType.add
MUL = mybir.AluOpType.mult
SIGMOID = mybir.ActivationFunctionType.Sigmoid


def build_program(w_scalars):
    """Build the per-core SPMD program. w_scalars = (w00, w01, w10, w11) when
    the per-patch weights are channel-uniform, else None (general path)."""
    nc = bacc.Bacc(None, target_bir_lowering=False, debug=False)

    # x is host-pre-gathered patch-major and zero-padded to 256 patches so
    # BOTH per-batch loads span 128 partitions (sub-128-partition HWDGE
    # transfers concentrate on 4 of the 16 SDMA engines).
    x_d = nc.declare_dram_parameter("x", [NB, 256, 4 * C], F32R, isOutput=False)
    e_d = nc.declare_dram_parameter("edge", [NB, N, C], F32R, isOutput=False)
    wlt_d = nc.declare_dram_parameter("wlt", [C, C], F32R, isOutput=False)
    wct_d = nc.declare_dram_parameter("wct", [C, CO], F32, isOutput=False)
    id_d = nc.declare_dram_parameter("ident", [128, 128], F32R, isOutput=False)
    clsc_d = nc.declare_dram_parameter("cls_cm", [128, 3, NB], F32, isOutput=False)
    if w_scalars is None:
        wqr_d = nc.declare_dram_parameter("wqr", [4, 128, C], F32, isOutput=False)
    out_d = nc.declare_dram_parameter("out", [NB, NOUT, CO], F32, isOutput=True)

    uniform_w = w_scalars is not None and len(set(w_scalars)) == 1
    # (tile index, rows, acm column offset)
    SPLITS = ((0, R1, 1), (1, R2, 1 + R1))

    with tile.TileContext(nc) as tc:
        with (
            tc.tile_pool(name="const", bufs=1) as cpool,
            tc.tile_pool(name="gx", bufs=6) as gxp,
            tc.tile_pool(name="ed", bufs=2) as edp,
            tc.tile_pool(name="apool", bufs=5) as ap,
            tc.tile_pool(name="work", bufs=2) as wk,
            tc.tile_pool(name="small", bufs=2) as sm,
            tc.tile_pool(name="cibp", bufs=1) as cibp,
            tc.tile_pool(name="acm", bufs=2) as acmp,
            tc.tile_pool(name="ost", bufs=2) as ostp,
            tc.tile_pool(name="psE", bufs=2, space="PSUM") as psE,
            tc.tile_pool(name="psC", bufs=1, space="PSUM") as psC,
            tc.tile_pool(name="psT", bufs=1, space="PSUM") as psT,
            tc.tile_pool(name="psF", bufs=2, space="PSUM") as psF,
        ):
            # ---- constants ----
            ones_f = cpool.tile([128, 1], F32)
            nc.vector.memset(ones_f[:], 1.0)
            ones_r = cpool.tile([128, 1], F32R)
            nc.vector.tensor_copy(ones_r[:], ones_f[:])

            ident_r = cpool.tile([128, 128], F32R)
            nc.sync.dma_start(ident_r[:], id_d[:])
            ident_bf = cpool.tile([128, 128], BF16)
            nc.vector.tensor_copy(ident_bf[:], ident_r[:])

            wlt_r = []
            for cch in range(3):
                t = cpool.tile([128, C], F32R, tag=f"wlt{cch}")
                nc.sync.dma_start(t[:], wlt_d[128 * cch : 128 * (cch + 1), :])
                wlt_r.append(t)

            wct_bf = []
            for cch in range(3):
                stg = ostp.tile([128, CO], F32, tag="ost0")
                nc.sync.dma_start(stg[:], wct_d[128 * cch : 128 * (cch + 1), :])
                t = cpool.tile([128, CO], BF16, tag=f"wct{cch}")
                nc.vector.tensor_copy(t[:], stg[:])
                wct_bf.append(t)

            if w_scalars is None:
                wqr_t = []
                for k in range(4):
                    t = cpool.tile([128, C], F32, tag=f"wqr{k}")
                    nc.sync.dma_start(t[:], wqr_d[k])
                    wqr_t.append(t)

            cls_cm = cpool.tile([128, 3, NB], F32)
            nc.sync.dma_start(cls_cm[:], clsc_d[:])
            cls_bf = cpool.tile([128, 3, NB], BF16)
            nc.vector.tensor_copy(cls_bf[:], cls_cm[:])

            group_list = [range(s, s + GRP) for s in range(0, NB, GRP)]
            for bs in group_list:
                glen = len(bs)
                g_t, a_t = {}, {}

                # -- sub-loop 1: loads + per-batch token sums --
                # one PSUM bank holds both the s-column collection (cols
                # 0:3*GRP) and the per-batch ci row (cols 3*GRP:3*GRP+C)
                chain_ps = psC.tile([128, 3 * GRP + C], F32, tag="chain")
                scolT = chain_ps[:, 0 : 3 * GRP].rearrange("p (c g) -> p c g", g=GRP)
                for b in bs:
                    gb = b - bs[0]
                    # all loads issue from SP (sync): its stream has no
                    # compute, so slot waits never block compute instructions.
                    g1 = gxp.tile([R1, 4, C], F32R, tag="g1")
                    nc.sync.dma_start(
                        g1[:], x_d[b, 0:128, :].rearrange("p (k c) -> p k c", k=4)
                    )
                    # g2 loads 128 padded patches; compute uses rows 0:68 only
                    g2f = gxp.tile([128, 4, C], F32R, tag="g2")
                    nc.sync.dma_start(
                        g2f[:], x_d[b, 128:256, :].rearrange("p (k c) -> p k c", k=4)
                    )
                    g_t[(b, 0)], g_t[(b, 1)] = g1, g2f

                    efold = edp.tile([128, 6 * C], F32R, tag="efold")
                    nc.sync.dma_start(
                        efold[:],
                        e_d[b, 0:768, :].rearrange("(p k) c -> p (k c)", p=128),
                    )
                    etl = edp.tile([17, C], F32R, tag="etl")
                    nc.gpsimd.dma_start(etl[:], e_d[b, 768:785, :])

                    # edge sums
                    es = psE.tile([1, C], F32, tag="es")
                    for k in range(6):
                        nc.tensor.matmul(
                            es[:], ones_r[:], efold[:, C * k : C * (k + 1)],
                            start=(k == 0), stop=False,
                        )
                    nc.tensor.matmul(
                        es[:], ones_r[0:17, :], etl[:], start=False, stop=True
                    )

                    # vertical pair sums A_q (f32r) + node sums
                    ns = psE.tile([1, C], F32, tag="ns")
                    first = True
                    for t_i, rn, _ in SPLITS:
                        at = ap.tile([rn, 2, C], F32R, tag=f"a{t_i}")
                        for q in range(2):
                            nc.vector.tensor_add(
                                at[:, q, :],
                                g_t[(b, t_i)][0:rn, q, :],
                                g_t[(b, t_i)][0:rn, 2 + q, :],
                            )
                        a_t[(b, t_i)] = at
                    for q in range(2):
                        for t_i, rn, _ in SPLITS:
                            nc.tensor.matmul(
                                ns[:], ones_r[0:rn, :], a_t[(b, t_i)][:, q, :],
                                start=first, stop=(q == 1 and t_i == 1),
                            )
                            first = False

                    # sigmoid means -> s_row; transpose into column gb of scolT
                    se = sm.tile([1, C], F32, tag="se")
                    nc.scalar.activation(se[:], es[:], SIGMOID, scale=1.0 / N)
                    sn = sm.tile([1, C], F32, tag="sn")
                    nc.scalar.activation(sn[:], ns[:], SIGMOID, scale=1.0 / HW)
                    s_row = sm.tile([1, C], F32, tag="srow")
                    nc.vector.tensor_add(s_row[:], se[:], sn[:])
                    for cch in range(3):
                        nc.tensor.matmul(
                            scolT[:, cch, gb : gb + 1],
                            s_row[:, 128 * cch : 128 * (cch + 1)],
                            ones_f[0:1, :],
                            start=True, stop=True,
                        )

                # -- group chain: s columns -> ci rows -> broadcasts up front --
                scolT_sb = sm.tile([128, 3, GRP], F32R, tag="scolsb")
                nc.scalar.copy(scolT_sb[:], scolT[:])

                cibs = []
                for gb in range(glen):
                    cip = chain_ps[0:1, 3 * GRP : 3 * GRP + C]
                    for cch in range(3):
                        nc.tensor.matmul(
                            cip[:], scolT_sb[:, cch, gb : gb + 1], wlt_r[cch][:],
                            start=(cch == 0), stop=(cch == 2),
                        )
                    ci_b = sm.tile([1, C], F32R, tag=f"cirow{gb}")
                    nc.scalar.copy(ci_b[:], cip[:])
                    cib = cibp.tile([128, C], F32R, tag=f"cib{gb}")
                    nc.gpsimd.partition_broadcast(cib[:], ci_b[:])
                    cibs.append(cib)

                # -- sub-loop 2: scores / pooled / transpose / final / store --
                for b in bs:
                    gb = b - bs[0]
                    cib = cibs[gb]

                    sig = {}
                    for t_i, rn, _ in SPLITS:
                        sa = sm.tile([rn, 4], F32, tag=f"sacc{t_i}")
                        for k in range(4):
                            scr = wk.tile([rn, C], BF16, tag=f"scr{t_i}")
                            nc.vector.scalar_tensor_tensor(
                                scr[:], g_t[(b, t_i)][0:rn, k, :], 1.0, cib[0:rn, :],
                                MUL, MUL, accum_out=sa[:, k : k + 1],
                            )
                        sg = sm.tile([rn, 4], F32, tag=f"sig{t_i}")
                        nc.scalar.activation(sg[:], sa[:], SIGMOID)
                        sig[t_i] = sg

                    pooled = {}
                    for t_i, rn, _ in SPLITS:
                        at = a_t[(b, t_i)]
                        if uniform_w:
                            # t01[:, q] = (sig[2q]+1) + (sig[2q+1]+1), w folded
                            w00 = w_scalars[0]
                            t01 = sm.tile([rn, 2], F32, tag=f"t01_{t_i}")
                            nc.vector.tensor_add(
                                t01[:], sig[t_i][:, 0:4:2], sig[t_i][:, 1:4:2]
                            )
                            if w00 != 1.0:
                                # (t01 + 2) * w00 == t01*w00 + 2*w00
                                nc.vector.tensor_scalar(
                                    t01[:], t01[:], w00, 2.0 * w00, MUL, ADD
                                )
                            else:
                                nc.vector.tensor_scalar_add(t01[:], t01[:], 2.0)
                            p0 = wk.tile([rn, C], F32, tag=f"p0_{t_i}")
                            nc.vector.tensor_scalar_mul(p0[:], at[:, 0, :], t01[:, 0:1])
                            pl = wk.tile([rn, C], BF16, tag=f"pl{t_i}")
                            nc.vector.scalar_tensor_tensor(
                                pl[:], at[:, 1, :], t01[:, 1:2], p0[:], MUL, ADD
                            )
                        elif w_scalars is not None:
                            sp = sm.tile([rn, 4], F32, tag=f"sp{t_i}")
                            nc.vector.tensor_scalar_add(sp[:], sig[t_i][:], 1.0)
                            wrow = sm.tile([rn, 4], F32, tag=f"wrow{t_i}")
                            for k in range(4):
                                nc.vector.memset(
                                    wrow[:, k : k + 1], float(w_scalars[k])
                                )
                            nc.vector.tensor_mul(sp[:], sp[:], wrow[:])
                            t01 = sm.tile([rn, 2], F32, tag=f"t01_{t_i}")
                            nc.vector.tensor_add(t01[:], sp[:, 0:4:2], sp[:, 1:4:2])
                            p0 = wk.tile([rn, C], F32, tag=f"p0_{t_i}")
                            nc.vector.tensor_scalar_mul(p0[:], at[:, 0, :], t01[:, 0:1])
                            pl = wk.tile([rn, C], BF16, tag=f"pl{t_i}")
                            nc.vector.scalar_tensor_tensor(
                                pl[:], at[:, 1, :], t01[:, 1:2], p0[:], MUL, ADD
                            )
                        else:
                            sp = sm.tile([rn, 4], F32, tag=f"sp{t_i}")
                            nc.vector.tensor_scalar_add(sp[:], sig[t_i][:], 1.0)
                            # general per-channel weights: m_q[rn, C], combine
                            mqs = []
                            for q in range(2):
                                m0 = wk.tile([rn, C], F32, tag=f"mq{q}{t_i}a")
                                nc.vector.tensor_scalar_mul(
                                    m0[:], wqr_t[2 * q][0:rn, :],
                                    sp[:, 2 * q : 2 * q + 1],
                                )
                                mq = wk.tile([rn, C], F32, tag=f"mq{q}{t_i}b")
                                nc.vector.scalar_tensor_tensor(
                                    mq[:], wqr_t[2 * q + 1][0:rn, :],
                                    sp[:, 2 * q + 1 : 2 * q + 2], m0[:], MUL, ADD,
                                )
                                mqs.append(mq)
                            p0 = wk.tile([rn, C], F32, tag=f"p0_{t_i}")
                            nc.vector.tensor_mul(p0[:], at[:, 0, :], mqs[0][:])
                            p1 = wk.tile([rn, C], F32, tag=f"p1_{t_i}")
                            nc.vector.tensor_mul(p1[:], at[:, 1, :], mqs[1][:])
                            pl = wk.tile([rn, C], BF16, tag=f"pl{t_i}")
                            nc.vector.tensor_add(pl[:], p0[:], p1[:])
                        pooled[t_i] = pl

                    # c-major A via bf16 PE transposes + cls column
                    a_cm = []
                    for cch in range(3):
                        tp2 = psT.tile([128, NPATCH], BF16, tag="tp2")
                        for t_i, rn, col in SPLITS:
                            nc.tensor.transpose(
                                tp2[:, col - 1 : col - 1 + rn],
                                pooled[t_i][:, 128 * cch : 128 * (cch + 1)],
                                ident_bf[0:rn, 0:rn],
                            )
                        acm = acmp.tile([128, NOUT], BF16, tag=f"acm{cch}")
                        nc.scalar.copy(acm[:, 0:1], cls_bf[:, cch, b : b + 1])
                        nc.scalar.copy(acm[:, 1:NOUT], tp2[:])
                        a_cm.append(acm)

                    # final matmul: out[row, co] = A_cm.T @ W_out_cls.T (bf16)
                    for rch, (r0, rn) in enumerate(((0, 128), (128, 69))):
                        stile = ostp.tile([128, CO], F32, tag=f"ost{rch}")
                        for nh in range(2):
                            fo = psF.tile([128, C], F32, tag="fo")
                            for cch in range(3):
                                nc.tensor.matmul(
                                    fo[0:rn, :],
                                    a_cm[cch][:, r0 : r0 + rn],
                                    wct_bf[cch][:, C * nh : C * (nh + 1)],
                                    start=(cch == 0), stop=(cch == 2),
                                )
                            # both halves on ACT so the store that follows in
                            # ACT's stream never waits on another engine
                            nc.scalar.copy(
                                stile[0:rn, C * nh : C * (nh + 1)], fo[0:rn, :]
                            )
                        # stores via SWDGE: its descriptor swizzle spreads
                        # partitions across all 16 SDMA engines
                        nc.gpsimd.dma_start(out_d[b, r0 : r0 + rn, :], stile[0:rn, :])

    nc.compile()
    return nc


def prepare(x, edge, W_lin, W_out_cls, weights):
    """Host-side prep shared by kernel() and the timing harness: returns
    (w_scalars, in_maps)."""
    x = np.ascontiguousarray(x, dtype=np.float32)
    edge = np.ascontiguousarray(edge, dtype=np.float32)
    # pre-gather nodes to patch-major [B, 196, 4C] (slot = 2p+q), zero-padded
    # to 256 patches so both per-batch loads span 128 partitions
    xg = np.zeros((B, 256, 4 * C), dtype=np.float32)
    xg[:, :NPATCH] = (
        x[:, 1:, :]
        .reshape(B, HP, 2, HP, 2, C)
        .transpose(0, 1, 3, 2, 4, 5)
        .reshape(B, NPATCH, 4 * C)
    )
    wlt = np.ascontiguousarray(np.asarray(W_lin).T, dtype=np.float32)
    wct = np.ascontiguousarray(np.asarray(W_out_cls).T, dtype=np.float32)
    w = np.asarray(weights, dtype=np.float32)

    c_uniform = bool(np.all(w == w[0:1]))
    w_scalars = tuple(float(v) for v in w[0].reshape(4)) if c_uniform else None

    ident = np.eye(128, dtype=np.float32)
    in_maps = []
    for core in range(NCORES):
        sl = slice(core * NB, (core + 1) * NB)
        cls_cm = np.ascontiguousarray(
            x[sl, 0, :].T.reshape(3, 128, NB).transpose(1, 0, 2), dtype=np.float32
        )
        m = {
            "x": xg[sl], "edge": edge[sl], "wlt": wlt, "wct": wct, "ident": ident,
            "cls_cm": cls_cm,
        }
        if w_scalars is None:
            wqr = np.empty((4, 128, C), dtype=np.float32)
            for q in range(2):
                for r in range(2):
                    wqr[2 * q + r] = np.broadcast_to(w[:, q, r], (128, C))
            m["wqr"] = wqr
        in_maps.append(m)
    return w_scalars, in_maps


def kernel(x, edge, W_lin, W_out_cls, weights):
    w_scalars, in_maps = prepare(x, edge, W_lin, W_out_cls, weights)
    nc = build_program(w_scalars)
    res = run_bass_kernel_spmd(nc, in_maps, list(range(NCORES)))
    out = np.concatenate([r["out"] for r in res.results], axis=0)
    return out
